# revision 16
# baseline (speedup 1.0000x reference)
"""GPTQ-style grouped-dequant linear on 8 Trainium2 cores.

out[m,n] = sum_k A[m,k] * (q[n,k] - zeros[n,k//128]) * scales[n,k//128] + bias[n]
M=2048, K=4096, N=4096, group=128.

Sharding: column-parallel - qweight/scales/zeros/bias split along N (512/core),
A replicated. Host does transport-layout prep: A transposed + bf16 (the same
rounding the kernel would do on-chip), q repacked to uint8, scales replicated
across the 128 k-partitions, plus per-group column sums of A (Ag) so the
zero-point term factors out of the dequant:

  out = sum_k A*(q*s)  -  sum_g Ag[m,g]*(z*s)[n,g]  +  bias[n]

Per core: W^T tiles are a single DVE mult per k-group (q * srep -> bf16),
consumed by PSUM-accumulated bf16 matmul chains (one per 128-row output
tile: a K=33 opener carrying the -Ag*(z*s) term and the bias, then 32
K=128 matmuls). Finish copies run on the Scalar engine. Zero-matmul warmup
fillers (kept alive by a scratch DRAM readout) bridge PE idle slots during
the DMA ramp so the HAM clock gate stays at 2.4 GHz throughout.
"""

import numpy as np
import ml_dtypes

import concourse.bass as bass
import concourse.mybir as mybir
import concourse.tile as tile
from concourse import bacc
from concourse.bass_utils import run_bass_kernel_spmd

P = 128
M, K, N = 2048, 4096, 4096
NCORES = 8
NS = N // NCORES          # 512 out-features per core
G = K // P                # 32 groups (group_size == P == 128)
MT = M // P               # 16 output row tiles
GC = 4                    # k-groups per DMA chunk
NLEAD = 7                 # accumulation chains live during the ramp phase
NWARM = 32                # initial HAM warmup matmuls (N=128 each)
FILLER = {0: 4, 1: 4, 2: 4, 3: 4, 4: 8, 5: 8, 6: 8, 7: 8,
          8: 8, 9: 8, 10: 8, 11: 8, 12: 8, 13: 8, 14: 8,
          15: 8}  # ramp filler matmuls after group g
CHUNKS = [4] * 8          # k-groups per DMA chunk

_cached = None


def _build():
    nc = bacc.Bacc("TRN2", target_bir_lowering=False, debug=False,
                   num_devices=NCORES, enable_partition_id=False)
    at = nc.dram_tensor("AT4", [MT, P, G, P], mybir.dt.bfloat16,
                        kind="ExternalInput")
    qt = nc.dram_tensor("q4", [P, G, NS], mybir.dt.uint8,
                        kind="ExternalInput")
    sr = nc.dram_tensor("srep", [P, G, NS], mybir.dt.bfloat16,
                        kind="ExternalInput")
    az = nc.dram_tensor("azt", [G + 1, M], mybir.dt.bfloat16,
                        kind="ExternalInput")
    zb = nc.dram_tensor("zbias", [G + 1, NS], mybir.dt.bfloat16,
                        kind="ExternalInput")
    out = nc.dram_tensor("out", [M, NS], mybir.dt.bfloat16,
                         kind="ExternalOutput")
    wscr = nc.dram_tensor("wscr", [P, 4], mybir.dt.float32,
                          kind="ExternalOutput")

    bf16, f32 = mybir.dt.bfloat16, mybir.dt.float32

    with tile.TileContext(nc) as tc:
        with (
            tc.tile_pool(name="const", bufs=1) as const,
            tc.tile_pool(name="qpool", bufs=1) as qpool,
            tc.tile_pool(name="reps", bufs=1) as reps,
            tc.tile_pool(name="wt", bufs=1) as wtp,
            tc.tile_pool(name="apool", bufs=8) as apool,
            tc.tile_pool(name="warmps", bufs=1, space="PSUM") as warmpool,
            tc.tile_pool(name="mpsum", bufs=7, space="PSUM") as mpsum,
            tc.tile_pool(name="opool", bufs=3) as opool,
        ):
            # --- HAM warmup: PE busy from t=0 so real matmuls run warm.
            # One long accumulation chain; the DRAM readout after the last
            # filler keeps it from being DCE'd.
            warm = const.tile([P, P], bf16, tag="warm")
            nc.gpsimd.memset(warm, 0.0)
            wps = warmpool.tile([P, NS], f32, tag="warmps")
            nwarm_total = NWARM + sum(FILLER.values())
            wcount = 0

            def warm_mm(n):
                nonlocal wcount
                for _ in range(n):
                    nc.tensor.matmul(wps[:, :P], warm[:], warm[:],
                                     start=(wcount == 0),
                                     stop=(wcount == nwarm_total - 1))
                    wcount += 1

            warm_mm(NWARM)

            # --- input DMAs, interleaved so W' groups + lead A tiles arrive
            # progressively ---
            q8s = qpool.tile([P, G, NS], mybir.dt.uint8, tag="q8s")
            srepT = reps.tile([P, G, NS], bf16, tag="srepT")
            aztT = const.tile([G + 1, M], bf16, tag="aztT")
            zbT = const.tile([G + 1, NS], bf16, tag="zbT")
            lead_ab = []
            qr, srr, atr = qt.ap(), sr.ap(), at.ap()
            g0 = 0
            for h, gc in enumerate(CHUNKS):
                g1 = g0 + gc
                nc.sync.dma_start(out=q8s[:, g0:g1, :], in_=qr[:, g0:g1, :])
                nc.sync.dma_start(out=srepT[:, g0:g1, :], in_=srr[:, g0:g1, :])
                if h == 0:
                    nc.sync.dma_start(out=aztT[:], in_=az.ap()[:])
                    nc.sync.dma_start(out=zbT[:], in_=zb.ap()[:])
                if h < NLEAD:
                    ab = apool.tile([P, G, P], bf16)
                    nc.sync.dma_start(out=ab[:, :G // 2, :],
                                      in_=atr[h, :, :G // 2, :])
                    nc.sync.dma_start(out=ab[:, G // 2:, :],
                                      in_=atr[h, :, G // 2:, :])
                    lead_ab.append(ab)
                g0 = g1

            def opener(mt, ps):
                # ps = -sum_g Ag[m,g]*(z*s)[n,g] + bias[n], via K=33 matmul
                nc.tensor.matmul(ps[:], aztT[:, mt * P:(mt + 1) * P], zbT[:],
                                 start=True, stop=False)

            def finish(mt, ps, last=False):
                ob = opool.tile([P, NS], bf16)
                if not last:
                    nc.scalar.copy(ob[:], ps[:])
                    nc.sync.dma_start(out=out.ap()[mt * P:(mt + 1) * P, :],
                                      in_=ob[:])
                    return
                # final tile: halves evacuated by ACT and DVE in parallel,
                # DMA'd separately, to shorten the kernel tail
                h = NS // 2
                nc.scalar.copy(ob[:, :h], ps[:, :h])
                nc.vector.tensor_copy(ob[:, h:], ps[:, h:])
                nc.sync.dma_start(out=out.ap()[mt * P:(mt + 1) * P, :h],
                                  in_=ob[:, :h])
                nc.sync.dma_start(out=out.ap()[mt * P:(mt + 1) * P, h:],
                                  in_=ob[:, h:])

            # --- phase 1: dequant each k-group (single DVE mult), consumed
            # by NLEAD concurrently-open PSUM accumulation chains; leads join
            # progressively (catch-up bursts) as their A strips arrive.
            # Already-joined leads advance BEFORE a new join's catch-up so a
            # late A tile can't head-of-line-block ready work. ---
            join_at = {0: 0, 1: 2, 2: 5, 3: 9, 4: 13, 5: 17, 6: 21}
            lead_ps = {}
            wts = []
            for g in range(G):
                wt = wtp.tile([P, NS], bf16, tag=f"wt{g}")
                nc.vector.tensor_tensor(wt[:], q8s[:, g, :], srepT[:, g, :],
                                        mybir.AluOpType.mult)
                wts.append(wt)
                for l in range(NLEAD):
                    if join_at[l] < g:
                        nc.tensor.matmul(lead_ps[l][:], lead_ab[l][:, g, :],
                                         wt[:], start=False,
                                         stop=(g == G - 1))
                for l in range(NLEAD):
                    if join_at[l] == g:
                        ps = mpsum.tile([P, NS], f32)
                        lead_ps[l] = ps
                        opener(l, ps)
                        for gc in range(g + 1):  # catch-up burst
                            nc.tensor.matmul(ps[:], lead_ab[l][:, gc, :],
                                             wts[gc][:], start=False,
                                             stop=(gc == G - 1))
                warm_mm(FILLER.get(g, 0))
            # readout keeping the whole warm/filler chain alive
            wsb = const.tile([P, 4], f32, tag="wsb")
            nc.vector.tensor_copy(wsb[:], wps[:, :4])
            nc.sync.dma_start(out=wscr.ap()[:], in_=wsb[:])

            for l in range(NLEAD):
                finish(l, lead_ps[l])

            # --- phase 2: remaining output tiles, dense back-to-back
            # matmuls with A tiles streaming underneath ---
            for mt in range(NLEAD, MT):
                ab = apool.tile([P, G, P], bf16)
                nc.sync.dma_start(out=ab[:, :G // 2, :],
                                  in_=atr[mt, :, :G // 2, :])
                nc.sync.dma_start(out=ab[:, G // 2:, :],
                                  in_=atr[mt, :, G // 2:, :])
                ps = mpsum.tile([P, NS], f32)
                opener(mt, ps)
                for g in range(G):
                    nc.tensor.matmul(ps[:], ab[:, g, :], wts[g][:],
                                     start=False, stop=(g == G - 1))
                finish(mt, ps, last=(mt == MT - 1))

    nc.compile()
    return nc


def _prep_inputs(A, qweight, scales, zeros, bias):
    bf = ml_dtypes.bfloat16
    # AT4[mt, p, g, j] = A[mt*128+j, g*128+p]  (layout permute + bf16)
    at4 = np.ascontiguousarray(
        A.reshape(MT, P, G, P).transpose(0, 3, 2, 1).astype(bf))
    # per-group column sums of A (for the factored zero-point term)
    ag = A.reshape(M, G, P).sum(axis=2)               # [M, G] f32
    azt = np.concatenate([ag.T, np.ones((1, M), np.float32)], axis=0)
    azt = np.ascontiguousarray(azt.astype(bf))        # [G+1, M]
    zs = -(zeros * scales)                            # [N, G] f32
    in_maps = []
    for c in range(NCORES):
        r = slice(c * NS, (c + 1) * NS)
        # q4[p, g, n] = q[n, g*128+p]
        q4 = np.ascontiguousarray(
            qweight[r].astype(np.uint8).T.reshape(G, P, NS).transpose(1, 0, 2))
        sT = scales[r].T.astype(bf)                   # [G, NS]
        zbias = np.concatenate([zs[r].T, bias[r][None, :]], axis=0)
        in_maps.append({
            "AT4": at4,
            "q4": q4,
            "srep": np.ascontiguousarray(
                np.broadcast_to(sT[None, :, :], (P, G, NS))),
            "azt": azt,
            "zbias": np.ascontiguousarray(zbias.astype(bf)),
        })
    return in_maps


def run(inputs, **spmd_kwargs):
    global _cached
    if _cached is None:
        _cached = _build()
    in_maps = _prep_inputs(**inputs)
    res = run_bass_kernel_spmd(_cached, in_maps, list(range(NCORES)),
                               **spmd_kwargs)
    outp = np.concatenate(
        [np.asarray(res.results[c]["out"]).astype(np.float32)
         for c in range(NCORES)], axis=1)
    return outp, res


def kernel(**inputs):
    return run(inputs)[0]


# revision 17
# speedup vs baseline: 1.0376x; 1.0376x over previous
"""GPTQ-style grouped-dequant linear on 8 Trainium2 cores.

out[m,n] = sum_k A[m,k] * (q[n,k] - zeros[n,k//128]) * scales[n,k//128] + bias[n]
M=2048, K=4096, N=4096, group=128.

Sharding: column-parallel - qweight/scales/zeros/bias split along N (512/core),
A replicated. Host does transport-layout prep: A transposed + bf16 (the same
rounding the kernel would do on-chip), q repacked to uint8, scales replicated
across the 128 k-partitions, plus per-group column sums of A (Ag) so the
zero-point term factors out of the dequant:

  out = sum_k A*(q*s)  -  sum_g Ag[m,g]*(z*s)[n,g]  +  bias[n]

Per core: W^T tiles are a single DVE mult per k-group (q * srep -> bf16),
consumed by PSUM-accumulated bf16 matmul chains (one per 128-row output
tile: a K=33 opener carrying the -Ag*(z*s) term and the bias, then 32
K=128 matmuls). Finish copies run on the Scalar engine. Zero-matmul warmup
fillers (kept alive by a scratch DRAM readout) bridge PE idle slots during
the DMA ramp so the HAM clock gate stays at 2.4 GHz throughout.
"""

import numpy as np
import ml_dtypes

import concourse.bass as bass
import concourse.mybir as mybir
import concourse.tile as tile
from concourse import bacc
from concourse.bass_utils import run_bass_kernel_spmd

P = 128
M, K, N = 2048, 4096, 4096
NCORES = 8
NS = N // NCORES          # 512 out-features per core
G = K // P                # 32 groups (group_size == P == 128)
MT = M // P               # 16 output row tiles
GC = 4                    # k-groups per DMA chunk
NLEAD = 7                 # accumulation chains live during the ramp phase
NWARM = 32                # initial HAM warmup matmuls (N=128 each)
FILLER = {0: 4, 1: 4, 2: 4, 3: 4, 4: 8, 5: 8, 6: 8, 7: 8,
          8: 8, 9: 8, 10: 6, 11: 6, 12: 4, 13: 4, 14: 2,
          15: 2}  # ramp filler matmuls after group g
CHUNKS = [4] * 8          # k-groups per DMA chunk

_cached = None


def _build():
    nc = bacc.Bacc("TRN2", target_bir_lowering=False, debug=False,
                   num_devices=NCORES)
    at = nc.dram_tensor("AT4", [MT, P, G, P], mybir.dt.bfloat16,
                        kind="ExternalInput")
    qt = nc.dram_tensor("q4", [P, G, NS], mybir.dt.uint8,
                        kind="ExternalInput")
    sr = nc.dram_tensor("srep", [P, G, NS], mybir.dt.bfloat16,
                        kind="ExternalInput")
    az = nc.dram_tensor("azt", [G + 1, M], mybir.dt.bfloat16,
                        kind="ExternalInput")
    zb = nc.dram_tensor("zbias", [G + 1, NS], mybir.dt.bfloat16,
                        kind="ExternalInput")
    out = nc.dram_tensor("out", [M, NS], mybir.dt.bfloat16,
                         kind="ExternalOutput")
    wscr = nc.dram_tensor("wscr", [P, 4], mybir.dt.float32,
                          kind="ExternalOutput")

    bf16, f32 = mybir.dt.bfloat16, mybir.dt.float32

    with tile.TileContext(nc) as tc:
        with (
            tc.tile_pool(name="const", bufs=1) as const,
            tc.tile_pool(name="qpool", bufs=1) as qpool,
            tc.tile_pool(name="reps", bufs=1) as reps,
            tc.tile_pool(name="wt", bufs=1) as wtp,
            tc.tile_pool(name="apool", bufs=8) as apool,
            tc.tile_pool(name="warmps", bufs=1, space="PSUM") as warmpool,
            tc.tile_pool(name="mpsum", bufs=7, space="PSUM") as mpsum,
            tc.tile_pool(name="opool", bufs=3) as opool,
        ):
            # --- HAM warmup: PE busy from t=0 so real matmuls run warm.
            # One long accumulation chain; the DRAM readout after the last
            # filler keeps it from being DCE'd.
            warm = const.tile([P, P], bf16, tag="warm")
            nc.gpsimd.memset(warm, 0.0)
            wps = warmpool.tile([P, NS], f32, tag="warmps")
            nwarm_total = NWARM + sum(FILLER.values())
            wcount = 0

            def warm_mm(n):
                nonlocal wcount
                for _ in range(n):
                    nc.tensor.matmul(wps[:, :P], warm[:], warm[:],
                                     start=(wcount == 0),
                                     stop=(wcount == nwarm_total - 1))
                    wcount += 1

            warm_mm(NWARM)

            # --- input DMAs, interleaved so W' groups + lead A tiles arrive
            # progressively ---
            q8s = qpool.tile([P, G, NS], mybir.dt.uint8, tag="q8s")
            srepT = reps.tile([P, G, NS], bf16, tag="srepT")
            aztT = const.tile([G + 1, M], bf16, tag="aztT")
            zbT = const.tile([G + 1, NS], bf16, tag="zbT")
            lead_ab = []
            qr, srr, atr = qt.ap(), sr.ap(), at.ap()
            g0 = 0
            for h, gc in enumerate(CHUNKS):
                g1 = g0 + gc
                nc.sync.dma_start(out=q8s[:, g0:g1, :], in_=qr[:, g0:g1, :])
                nc.sync.dma_start(out=srepT[:, g0:g1, :], in_=srr[:, g0:g1, :])
                if h == 0:
                    nc.sync.dma_start(out=aztT[:], in_=az.ap()[:])
                    nc.sync.dma_start(out=zbT[:], in_=zb.ap()[:])
                if h < NLEAD:
                    ab = apool.tile([P, G, P], bf16)
                    nc.sync.dma_start(out=ab[:, :G // 2, :],
                                      in_=atr[h, :, :G // 2, :])
                    lead_ab.append(ab)
                if 0 <= h - 3 < NLEAD:  # deferred second half of lead h-3
                    nc.sync.dma_start(out=lead_ab[h - 3][:, G // 2:, :],
                                      in_=atr[h - 3, :, G // 2:, :])
                g0 = g1
            for l in range(len(CHUNKS) - 3, NLEAD):  # remaining second halves
                nc.sync.dma_start(out=lead_ab[l][:, G // 2:, :],
                                  in_=atr[l, :, G // 2:, :])

            def opener(mt, ps):
                # ps = -sum_g Ag[m,g]*(z*s)[n,g] + bias[n], via K=33 matmul
                nc.tensor.matmul(ps[:], aztT[:, mt * P:(mt + 1) * P], zbT[:],
                                 start=True, stop=False)

            def finish(mt, ps, last=False):
                ob = opool.tile([P, NS], bf16)
                if not last:
                    nc.scalar.copy(ob[:], ps[:])
                    nc.sync.dma_start(out=out.ap()[mt * P:(mt + 1) * P, :],
                                      in_=ob[:])
                    return
                # final tile: halves evacuated by ACT and DVE in parallel,
                # DMA'd separately, to shorten the kernel tail
                h = NS // 2
                nc.scalar.copy(ob[:, :h], ps[:, :h])
                nc.vector.tensor_copy(ob[:, h:], ps[:, h:])
                nc.sync.dma_start(out=out.ap()[mt * P:(mt + 1) * P, :h],
                                  in_=ob[:, :h])
                nc.sync.dma_start(out=out.ap()[mt * P:(mt + 1) * P, h:],
                                  in_=ob[:, h:])

            # --- phase 1: dequant each k-group (single DVE mult), consumed
            # by NLEAD concurrently-open PSUM accumulation chains; leads join
            # progressively (catch-up bursts) as their A strips arrive.
            # Already-joined leads advance BEFORE a new join's catch-up so a
            # late A tile can't head-of-line-block ready work. ---
            join_at = {0: 0, 1: 2, 2: 5, 3: 9, 4: 13, 5: 17, 6: 21}
            lead_ps = {}
            wts = []
            for g in range(G):
                wt = wtp.tile([P, NS], bf16, tag=f"wt{g}")
                nc.vector.tensor_tensor(wt[:], q8s[:, g, :], srepT[:, g, :],
                                        mybir.AluOpType.mult)
                wts.append(wt)
                for l in range(NLEAD):
                    if join_at[l] < g:
                        nc.tensor.matmul(lead_ps[l][:], lead_ab[l][:, g, :],
                                         wt[:], start=False,
                                         stop=(g == G - 1))
                for l in range(NLEAD):
                    if join_at[l] == g:
                        ps = mpsum.tile([P, NS], f32)
                        lead_ps[l] = ps
                        opener(l, ps)
                        for gc in range(g + 1):  # catch-up burst
                            nc.tensor.matmul(ps[:], lead_ab[l][:, gc, :],
                                             wts[gc][:], start=False,
                                             stop=(gc == G - 1))
                warm_mm(FILLER.get(g, 0))
            # readout keeping the whole warm/filler chain alive
            wsb = const.tile([P, 4], f32, tag="wsb")
            nc.vector.tensor_copy(wsb[:], wps[:, :4])
            nc.sync.dma_start(out=wscr.ap()[:], in_=wsb[:])

            for l in range(NLEAD):
                finish(l, lead_ps[l])

            # --- phase 2: remaining output tiles, dense back-to-back
            # matmuls with A tiles streaming underneath ---
            for mt in range(NLEAD, MT):
                ab = apool.tile([P, G, P], bf16)
                nc.sync.dma_start(out=ab[:, :G // 2, :],
                                  in_=atr[mt, :, :G // 2, :])
                nc.sync.dma_start(out=ab[:, G // 2:, :],
                                  in_=atr[mt, :, G // 2:, :])
                ps = mpsum.tile([P, NS], f32)
                opener(mt, ps)
                for g in range(G):
                    nc.tensor.matmul(ps[:], ab[:, g, :], wts[g][:],
                                     start=False, stop=(g == G - 1))
                finish(mt, ps, last=(mt == MT - 1))

    nc.compile()
    return nc


def _prep_inputs(A, qweight, scales, zeros, bias):
    bf = ml_dtypes.bfloat16
    # AT4[mt, p, g, j] = A[mt*128+j, g*128+p]  (layout permute + bf16)
    at4 = np.ascontiguousarray(
        A.reshape(MT, P, G, P).transpose(0, 3, 2, 1).astype(bf))
    # per-group column sums of A (for the factored zero-point term)
    ag = A.reshape(M, G, P).sum(axis=2)               # [M, G] f32
    azt = np.concatenate([ag.T, np.ones((1, M), np.float32)], axis=0)
    azt = np.ascontiguousarray(azt.astype(bf))        # [G+1, M]
    zs = -(zeros * scales)                            # [N, G] f32
    in_maps = []
    for c in range(NCORES):
        r = slice(c * NS, (c + 1) * NS)
        # q4[p, g, n] = q[n, g*128+p]
        q4 = np.ascontiguousarray(
            qweight[r].astype(np.uint8).T.reshape(G, P, NS).transpose(1, 0, 2))
        sT = scales[r].T.astype(bf)                   # [G, NS]
        zbias = np.concatenate([zs[r].T, bias[r][None, :]], axis=0)
        in_maps.append({
            "AT4": at4,
            "q4": q4,
            "srep": np.ascontiguousarray(
                np.broadcast_to(sT[None, :, :], (P, G, NS))),
            "azt": azt,
            "zbias": np.ascontiguousarray(zbias.astype(bf)),
        })
    return in_maps


def run(inputs, **spmd_kwargs):
    global _cached
    if _cached is None:
        _cached = _build()
    in_maps = _prep_inputs(**inputs)
    res = run_bass_kernel_spmd(_cached, in_maps, list(range(NCORES)),
                               **spmd_kwargs)
    outp = np.concatenate(
        [np.asarray(res.results[c]["out"]).astype(np.float32)
         for c in range(NCORES)], axis=1)
    return outp, res


def kernel(**inputs):
    return run(inputs)[0]
